# revision 20
# baseline (speedup 1.0000x reference)
"""MoDA attention Trainium2 kernel.

nn_MoDAAttention: B=2, T=2048, D=1024, HQ=16, HK=4, HD=64, LP=8.

Sharding: 8 cores = (batch 2) x (kv-head 4). Each core handles one batch row
and one kv head (4 query heads, GQA group). Depth KV cache shards along HK.
o_proj weight row-sharded by head group; partial outputs summed on host.

Per-core plan (all on one NeuronCore, Tile-scheduled):
  phase 1: projections  QT/KT/VT/kdT/vdT = W.T @ xT   (fp32r matmuls)
  phase 2a: depth attention scores in transposed orientation:
            prod_l = QT_bf16 * KdT_l (DVE, bf16 2x), s_dep^T = ones-selector
            matmul over d (PE), exp (ACT) -> p_dep^T; transpose to p_nat;
            depth PV: p_nat (bcast) * Vd_dmaj -> reduce over l -> O_dep (nat).
  phase 2b: per q-chunk of 512: S^T = K^T.T @ Q^T (row-packed pairs of K=64
            matmuls), exp -> P^T bf16, PV: [V|1].T @ P^T accumulates
            [O^T; den] in PSUM over k-tiles; depth den + O_dep^T folded into
            the same PSUM via selector/transpose matmuls; reciprocal ->
            K=1 broadcast matmul -> H^T = O^T * (1/den).
  phase 3: o_proj: Y = H^T.T @ Wo (fp32r), DMA out.

kernel(**inputs) takes FULL inputs, shards + lays out on host (numpy),
runs SPMD on cores 0-7, gathers on host.
"""

import sys
from contextlib import ExitStack

import numpy as np

for _p in ("/opt/trn_rl_repo", "/opt/trn_rl_repo/concourse"):
    if _p not in sys.path:
        sys.path.insert(0, _p)

import ml_dtypes

import concourse.bass as bass
import concourse.mybir as mybir
import concourse.tile as tile
from concourse import bacc
from concourse.masks import make_identity

F32 = mybir.dt.float32
F32R = mybir.dt.float32r
BF16 = mybir.dt.float16
BF = np.float16

B, T, D, HQ, HK, HD, LP = 2, 2048, 1024, 16, 4, 64, 8
G = HQ // HK            # q heads per core = 4
NKI = T // 128          # 16 k-tiles
NQC = T // 512          # 4 q-chunks
NDC = D // 128          # 8 D chunks
SCALE = HD ** -0.5      # 1/8


DEBUG_OUTPUTS = False


def r(ap):
    return ap.bitcast(F32R)


def build_nc():
    nc = bacc.Bacc("TRN2", target_bir_lowering=False, debug=False)

    # ---- DRAM I/O ----
    xT = nc.dram_tensor("xT", [D, T], F32R, kind="ExternalInput")
    w_proj = nc.dram_tensor("w_proj", [NDC, 128, 512], F32R, kind="ExternalInput")
    wo = nc.dram_tensor("wo", [2, 128, D], F32R, kind="ExternalInput")
    kdT = nc.dram_tensor("kdT", [LP, 128, T], BF16, kind="ExternalInput")
    vd_dmaj = nc.dram_tensor("vd_dmaj", [T, LP * HD], BF16, kind="ExternalInput")
    trimask = nc.dram_tensor("trimask", [128, 128], BF16, kind="ExternalInput")
    sel = nc.dram_tensor("sel", [LP, 128, 16], BF16, kind="ExternalInput")
    densel = nc.dram_tensor("densel", [2, 16, 66], F32R, kind="ExternalInput")
    identity = nc.dram_tensor("identity", [128, 128], F32R, kind="ExternalInput")
    ones64 = nc.dram_tensor("ones64", [1, 64], F32R, kind="ExternalInput")

    y_out = nc.dram_tensor("y", [T, D], F32, kind="ExternalOutput")
    kvT_out = nc.dram_tensor("kvT", [128, T], F32, kind="ExternalOutput")
    if DEBUG_OUTPUTS:
        dbg_pdepT = nc.dram_tensor("dbg_pdepT", [16, T], F32, kind="ExternalOutput")
        dbg_odep = nc.dram_tensor("dbg_odep", [128, NKI * HD], F32,
                                  kind="ExternalOutput")
        dbg_HT = nc.dram_tensor("dbg_HT", [128, T], F32, kind="ExternalOutput")
        dbg_pt = nc.dram_tensor("dbg_pt", [128, 512], F32, kind="ExternalOutput")
        dbg_qt = nc.dram_tensor("dbg_qt", [128, T], F32, kind="ExternalOutput")
        dbg_pnat = nc.dram_tensor("dbg_pnat", [128, NKI * 16], F32,
                                  kind="ExternalOutput")
        dbg_pv1 = nc.dram_tensor("dbg_pv1", [66, 512], F32, kind="ExternalOutput")
        dbg_pv2 = nc.dram_tensor("dbg_pv2", [66, 512], F32, kind="ExternalOutput")
        dbg_bcs = nc.dram_tensor("dbg_bcs", [64, 512], F32, kind="ExternalOutput")
        dbg_rec = nc.dram_tensor("dbg_rec", [1, 512], F32, kind="ExternalOutput")
        dbg_den = nc.dram_tensor("dbg_den", [1, 512], F32, kind="ExternalOutput")

    with tile.TileContext(nc) as tc, ExitStack() as ctx:
        const = ctx.enter_context(tc.tile_pool(name="const", bufs=1))
        persist = ctx.enter_context(tc.tile_pool(name="persist", bufs=1))

        # constants
        w_sb = [const.tile([128, 512], F32R, name=f"w{dc}") for dc in range(NDC)]
        for dc in range(NDC):
            nc.sync.dma_start(w_sb[dc][:], w_proj[dc])
        wo_sb = [const.tile([128, D], F32R, name=f"wo{p}") for p in range(2)]
        for p in range(2):
            nc.sync.dma_start(wo_sb[p][:], wo[p])
        trimask_sb = const.tile([128, 128], BF16)
        nc.sync.dma_start(trimask_sb[:], trimask[:])
        sel_sb = [const.tile([128, 16], BF16, name=f"sel{l}") for l in range(LP)]
        for l in range(LP):
            nc.sync.dma_start(sel_sb[l][:], sel[l])
        densel_sb = [const.tile([16, 66], F32R, name=f"densel{hp}") for hp in range(2)]
        for hp in range(2):
            nc.sync.dma_start(densel_sb[hp][:], densel[hp])
        ones64_sb = const.tile([1, 64], F32R)
        nc.sync.dma_start(ones64_sb[:], ones64[:])
        ident = const.tile([128, 128], F32R)
        nc.sync.dma_start(ident[:], identity[:])

        # persistent intermediates
        QT = [persist.tile([128, T], F32R, name=f"QT{p}") for p in range(2)]
        QTbf = [persist.tile([128, T], BF16, name=f"QTbf{p}") for p in range(2)]
        vk_sb = persist.tile([128, T], F32R, name="vk")      # rows 0:64 V^T, 64:128 K^T
        kv_sb = persist.tile([128, T], F32, name="kv")      # rows 0:64 kd^T, 64:128 vd^T
        ktlow = persist.tile([64, T], F32R, name="ktlow")    # K^T copy at partitions 0-63
        vones = [persist.tile([128, 66], BF16, name=f"vones{ki}") for ki in range(NKI)]
        pdepT = [persist.tile([16, T], F32R, name=f"pdepT{p}") for p in range(2)]
        p_nat = [persist.tile([128, NKI * 16], BF16, name=f"pnat{p}") for p in range(2)]
        odep = [persist.tile([128, NKI * HD], F32R, name=f"odep{h}") for h in range(G)]
        HT = [persist.tile([128, T], F32R, name=f"HT{p}") for p in range(2)]

        # ---------------- phase 1: projections ----------------
        with tc.tile_pool(name="xpool", bufs=3) as xpool, \
             tc.tile_pool(name="ppsum", bufs=2, space="PSUM") as ppsum:
            for tci in range(NQC):
                psums = [ppsum.tile([128, 512], F32, name=f"pj{m}") for m in range(4)]
                for dc in range(NDC):
                    xt = xpool.tile([128, 512], F32R)
                    nc.sync.dma_start(
                        xt[:], xT[dc * 128:(dc + 1) * 128, tci * 512:(tci + 1) * 512])
                    for m in range(4):
                        nc.tensor.matmul(
                            psums[m][:], w_sb[dc][:, m * 128:(m + 1) * 128],
                            xt[:], start=(dc == 0), stop=(dc == NDC - 1))
                ts = slice(tci * 512, (tci + 1) * 512)
                nc.scalar.copy(QT[0][:, ts], psums[0][:])
                nc.scalar.copy(QT[1][:, ts], psums[1][:])
                nc.vector.tensor_copy(vk_sb[:, ts], psums[2][:])
                nc.vector.tensor_copy(kv_sb[:, ts], psums[3][:])

        # kd/vd out; K^T dup to low partitions; bf16 Q
        nc.sync.dma_start(kvT_out[:], kv_sb[:])
        nc.sync.dma_start(ktlow[:], vk_sb[64:128, :])
        for p in range(2):
            nc.vector.tensor_copy(QTbf[p][:], QT[p][:])

        # [V|1] tiles: transpose V^T chunks, cast to bf16, ones column
        with tc.tile_pool(name="vtp", bufs=4, space="PSUM") as vtp:
            for ki in range(NKI):
                tp = vtp.tile([128, 64], F32)
                nc.tensor.matmul(r(tp[:]), vk_sb[0:64, ki * 128:(ki + 1) * 128],
                                 ident[0:64, 0:64], is_transpose=True, start=True,
                                 stop=True)
                nc.scalar.copy(vones[ki][:, 0:64], tp[:])
                nc.vector.memset(vones[ki][:, 64:65], 1.0)
                nc.vector.memset(vones[ki][:, 65:66], 0.0)

        # ---------------- phase 2a: depth attention ----------------
        with tc.tile_pool(name="kdpool", bufs=8) as kdpool, \
             tc.tile_pool(name="dprod", bufs=4) as dprod, \
             tc.tile_pool(name="dpsum", bufs=2, space="PSUM") as dpsum:
            kdt_sb = [kdpool.tile([128, T], BF16, name="kdt") for _ in range(LP)]
            for l in range(LP):
                nc.sync.dma_start(kdt_sb[l][:], kdT[l])
            for pr in range(2):
                for qci in range(NQC):
                    qs = slice(qci * 512, (qci + 1) * 512)
                    sdep = dpsum.tile([16, 512], F32, name="sdep")
                    for l in range(LP):
                        prod = dprod.tile([128, 512], BF16, name="prod")
                        nc.vector.tensor_tensor(
                            prod[:], QTbf[pr][:, qs], kdt_sb[l][:, qs],
                            op=mybir.AluOpType.mult)
                        nc.tensor.matmul(sdep[:], sel_sb[l][:], prod[:],
                                         start=(l == 0), stop=(l == LP - 1))
                    nc.scalar.activation(pdepT[pr][:, qs], sdep[:],
                                         mybir.ActivationFunctionType.Exp, scale=SCALE)
                    # transpose p_dep^T -> p_nat (4 chunks of 128 cols)
                    pn = dpsum.tile([128, 64], F32, name="pn")
                    for j in range(4):
                        cs = slice(qci * 512 + j * 128, qci * 512 + (j + 1) * 128)
                        nc.tensor.matmul(
                            r(pn[:, j * 16:(j + 1) * 16]), pdepT[pr][:, cs],
                            ident[0:16, 0:16], is_transpose=True, start=True,
                            stop=True, skip_group_check=True)
                    nc.scalar.copy(p_nat[pr][:, qci * 64:(qci + 1) * 64], pn[:])

        # depth PV: O_dep[t, d] = sum_l p[t,l] * Vd[t,l,d]
        with tc.tile_pool(name="vdpool", bufs=3) as vdpool, \
             tc.tile_pool(name="dvprod", bufs=4) as dvprod:
            for tt in range(NKI):
                vd = vdpool.tile([128, LP * HD], BF16, name="vd")
                nc.sync.dma_start(vd[:], vd_dmaj[tt * 128:(tt + 1) * 128, :])
                vdv = vd[:].rearrange("p (d l) -> p d l", l=LP)
                for h in range(G):
                    pr, hp = divmod(h, 2)
                    base = tt * 16 + hp * 8
                    pb = (p_nat[pr][:, base:base + 8]
                          .rearrange("p (o l) -> p o l", o=1)
                          .to_broadcast((128, HD, LP)))
                    pv = dvprod.tile([128, LP * HD], BF16, name="pv")
                    pvv = pv[:].rearrange("p (d l) -> p d l", l=LP)
                    nc.vector.tensor_tensor(pvv, pb, vdv, op=mybir.AluOpType.mult)
                    with nc.allow_low_precision(reason="f32r out, fp32 accum"):
                        nc.vector.tensor_reduce(
                            odep[h][:, tt * HD:(tt + 1) * HD], pvv,
                            axis=mybir.AxisListType.X, op=mybir.AluOpType.add)

        # ---------------- phase 2b: sequence attention ----------------
        with tc.tile_pool(name="pvps", bufs=1, space="PSUM") as pvps, \
             tc.tile_pool(name="stps", bufs=3, space="PSUM") as stps, \
             tc.tile_pool(name="bcps", bufs=1, space="PSUM") as bcps, \
             tc.tile_pool(name="ptpool", bufs=6) as ptpool, \
             tc.tile_pool(name="small", bufs=2) as small:
            for qci in range(NQC):
                qs = slice(qci * 512, (qci + 1) * 512)
                n_ki = 4 * qci + 4
                pv = [pvps.tile([66, 512], F32, name=f"pv{h}") for h in range(G)]
                for ki in range(n_ki):
                    ks = slice(ki * 128, (ki + 1) * 128)
                    pts = []
                    for pr in range(2):
                        st0 = stps.tile([128, 512], F32, name="st")
                        st1 = stps.tile([128, 512], F32, name="st")
                        nc.tensor.matmul(st0[:], ktlow[:, ks],
                                         QT[pr][0:64, qs], start=True, stop=True,
                                         tile_position=(0, 0))
                        nc.tensor.matmul(st1[:], vk_sb[64:128, ks],
                                         QT[pr][64:128, qs], start=True, stop=True,
                                         tile_position=(64, 0))
                        pts.append(st0)
                        pts.append(st1)
                    for h in range(G):
                        st = pts[h]
                        pt = ptpool.tile([128, 512], BF16, name="pt")
                        left = ki * 128 - qci * 512
                        if left >= 0:  # diagonal block present
                            if left > 0:
                                nc.vector.memset(pt[:, 0:left], 0.0)
                            nc.scalar.activation(
                                pt[:, left:512], st[:, left:512],
                                mybir.ActivationFunctionType.Exp, scale=SCALE)
                            nc.vector.tensor_tensor(
                                pt[:, left:left + 128], pt[:, left:left + 128],
                                trimask_sb[:], op=mybir.AluOpType.mult)
                        else:
                            nc.scalar.activation(
                                pt[:], st[:], mybir.ActivationFunctionType.Exp,
                                scale=SCALE)
                        if DEBUG_OUTPUTS and qci == 0 and ki == 0 and h == 0:
                            dbg_pt_sb = small.tile([128, 512], F32, name="dbgpt")
                            nc.vector.tensor_copy(dbg_pt_sb[:], pt[:])
                            nc.sync.dma_start(dbg_pt[:], dbg_pt_sb[:])
                        nc.tensor.matmul(pv[h][0:66, :], vones[ki][:], pt[:],
                                         start=(ki == 0), stop=False,
                                         skip_group_check=True)
                if DEBUG_OUTPUTS and qci == 0:
                    dbg_pv1_sb = small.tile([66, 512], F32, name="dbgpv")
                    nc.vector.tensor_copy(dbg_pv1_sb[:], pv[0][:])
                    nc.sync.dma_start(dbg_pv1[:], dbg_pv1_sb[:])
                # fold depth denominator + O_dep^T into the PV psum
                for h in range(G):
                    pr, hp = divmod(h, 2)
                    nc.tensor.matmul(pv[h][0:66, :], densel_sb[hp][:],
                                     pdepT[pr][:, qs], start=False, stop=False,
                                     skip_group_check=True)
                    for j in range(4):
                        tt = qci * 4 + j
                        nc.tensor.matmul(
                            r(pv[h][0:64, j * 128:(j + 1) * 128]),
                            odep[h][:, tt * HD:(tt + 1) * HD], ident[:],
                            is_transpose=True, start=False, stop=(j == 3),
                            skip_group_check=True)
                if DEBUG_OUTPUTS and qci == 0:
                    dbg_pv2_sb = small.tile([66, 512], F32, name="dbgpv")
                    nc.vector.tensor_copy(dbg_pv2_sb[:], pv[0][:])
                    nc.sync.dma_start(dbg_pv2[:], dbg_pv2_sb[:])
                # normalize: H^T = O^T / den (all on partition 64: engines
                # cannot cross partitions; the K=1 matmul broadcasts back down)
                for h in range(G):
                    pr, hp = divmod(h, 2)
                    rec = small.tile([1, 512], F32, name="rec")
                    scr = small.tile([1, 512], F32, name="scr")
                    dsb = small.tile([65, 512], F32, name="dsb")
                    dlow = small.tile([1, 512], F32, name="dlow")
                    # den lives on partition 64 (PV psum row 64); engines can't
                    # cross partitions and the custom recip needs base 0, so
                    # hop: ACT copy (psum->sbuf, same partition), DMA down.
                    nc.scalar.copy(dsb[64:65, :], pv[h][64:65, :])
                    nc.sync.dma_start(dlow[:], dsb[64:65, :])
                    nc.vector.reciprocal_approx_accurate(
                        rec[:], dlow[:], scr[:])
                    if DEBUG_OUTPUTS and qci == 0 and h == 0:
                        nc.sync.dma_start(dbg_rec[:], rec[:])
                        nc.sync.dma_start(dbg_den[:], dlow[:])
                    rec_r = small.tile([1, 512], F32R, name="recr")
                    nc.vector.tensor_copy(rec_r[:], rec[:])
                    bc = bcps.tile([64, 512], F32, name="bc")
                    nc.tensor.matmul(bc[:], ones64_sb[:], rec_r[:],
                                     start=True, stop=True)
                    bcs = small.tile([64, 512], F32, name="bcs")
                    nc.scalar.copy(bcs[:], bc[:])
                    if DEBUG_OUTPUTS and qci == 0 and h == 0:
                        nc.sync.dma_start(dbg_bcs[:], bcs[:])
                    if hp == 0:
                        nc.vector.tensor_tensor(
                            HT[pr][0:64, qs], pv[h][0:64, :],
                            bcs[:], op=mybir.AluOpType.mult)
                    else:
                        # odd head: DVE writes base-0 temp; DMA shifts to
                        # partitions 64-127 (engines can't cross partitions)
                        httmp = small.tile([64, 512], F32R, name="httmp")
                        nc.vector.tensor_tensor(
                            httmp[:], pv[h][0:64, :], bcs[:],
                            op=mybir.AluOpType.mult)
                        nc.sync.dma_start(HT[pr][64:128, qs], httmp[:])

        if DEBUG_OUTPUTS:
            dbg1 = persist.tile([16, T], F32, name="dbg1")
            nc.vector.tensor_copy(dbg1[:], pdepT[0][:])
            nc.sync.dma_start(dbg_pdepT[:], dbg1[:])
            dbg2 = persist.tile([128, NKI * HD], F32, name="dbg2")
            nc.vector.tensor_copy(dbg2[:], odep[0][:])
            nc.sync.dma_start(dbg_odep[:], dbg2[:])
            dbg3 = persist.tile([128, T], F32, name="dbg3")
            nc.vector.tensor_copy(dbg3[:], HT[0][:])
            nc.sync.dma_start(dbg_HT[:], dbg3[:])
            dbg4 = persist.tile([128, T], F32, name="dbg4")
            nc.vector.tensor_copy(dbg4[:], QT[0][:])
            nc.sync.dma_start(dbg_qt[:], dbg4[:])
            dbg5 = persist.tile([128, NKI * 16], F32, name="dbg5")
            nc.vector.tensor_copy(dbg5[:], p_nat[0][:])
            nc.sync.dma_start(dbg_pnat[:], dbg5[:])

        # ---------------- phase 3: o_proj ----------------
        with tc.tile_pool(name="ypsum", bufs=4, space="PSUM") as ypsum, \
             tc.tile_pool(name="ypool", bufs=3) as ypool:
            for tt in range(NKI):
                cs = slice(tt * 128, (tt + 1) * 128)
                ysb = ypool.tile([128, D], F32, name="ysb")
                for ec in range(2):
                    es = slice(ec * 512, (ec + 1) * 512)
                    yp = ypsum.tile([128, 512], F32, name="yp")
                    for p in range(2):
                        nc.tensor.matmul(yp[:], HT[p][:, cs], wo_sb[p][:, es],
                                         start=(p == 0), stop=(p == 1))
                    if ec == 0:
                        nc.scalar.copy(ysb[:, es], yp[:])
                    else:
                        nc.vector.tensor_copy(ysb[:, es], yp[:])
                nc.sync.dma_start(y_out[cs, :], ysb[:])

    nc.compile()
    return nc


_NC_CACHE = None


def _get_nc():
    global _NC_CACHE
    if _NC_CACHE is None:
        _NC_CACHE = build_nc()
    return _NC_CACHE


def _make_core_inputs(x, K_depth, V_depth, Wq, Wk, Wv, Wkd, Wvd, Wo, b, h):
    xT = np.ascontiguousarray(x[b].T)                       # (D, T)
    wq = Wq[:, h * G * HD:(h + 1) * G * HD]                 # (D, 256)
    wk = Wk[:, h * HD:(h + 1) * HD]
    wv = Wv[:, h * HD:(h + 1) * HD]
    wkd = Wkd[:, h * HD:(h + 1) * HD]
    wvd = Wvd[:, h * HD:(h + 1) * HD]
    # m0=[q0|q1] m1=[q2|q3] m2=[V|K] m3=[kd|vd]
    w_stack = np.concatenate([wq, wv, wk, wkd, wvd], axis=1)  # (D, 512)
    w_proj = np.ascontiguousarray(w_stack.reshape(NDC, 128, 512), dtype=np.float32)
    wo = np.ascontiguousarray(
        Wo[h * G * HD:(h + 1) * G * HD, :].reshape(2, 128, D), dtype=np.float32)
    kd = K_depth[b, h].reshape(T, LP, HD)
    kdT_l = np.transpose(kd, (1, 2, 0))                     # (LP, HD, T)
    kdT = np.concatenate([kdT_l, kdT_l], axis=1).astype(BF)  # (LP, 128, T)
    vd = V_depth[b, h].reshape(T, LP, HD)
    vd_dmaj = np.ascontiguousarray(
        np.transpose(vd, (0, 2, 1)).reshape(T, LP * HD)).astype(BF)
    return dict(xT=xT, w_proj=w_proj, wo=wo, kdT=np.ascontiguousarray(kdT),
                vd_dmaj=vd_dmaj)


def _const_inputs():
    k = np.arange(128)[:, None]
    q = np.arange(128)[None, :]
    trimask = (q >= k).astype(BF)                            # (128, 128)
    sel = np.zeros((LP, 128, 16), dtype=BF)
    for l in range(LP):
        sel[l, 0:64, l] = 1.0
        sel[l, 64:128, 8 + l] = 1.0
    densel = np.zeros((2, 16, 66), dtype=np.float32)
    for hp in range(2):
        densel[hp, hp * 8:(hp + 1) * 8, 64] = 1.0
    ones64 = np.ones((1, 64), dtype=np.float32)
    identity = np.eye(128, dtype=np.float32)
    return dict(trimask=trimask, sel=sel, densel=densel, ones64=ones64,
                identity=identity)


def _ensure_ntff_hook():
    """Provide the antenv.axon_hooks shim (missing in this image) so
    run_bass_kernel_spmd(trace=True) can capture NTFF profiles via the
    ctypes hook in libaxon_pjrt.so (mirrors trn_agent_boot.trn_boot)."""
    import types

    try:
        from antenv.axon_hooks import get_axon_ntff_profile_hook  # noqa: F401
        return
    except ImportError:
        pass
    import antenv

    mod = types.ModuleType("antenv.axon_hooks")
    mod._hook = None

    def set_axon_ntff_profile_hook(h):
        mod._hook = h

    def get_axon_ntff_profile_hook():
        return mod._hook

    mod.set_axon_ntff_profile_hook = set_axon_ntff_profile_hook
    mod.get_axon_ntff_profile_hook = get_axon_ntff_profile_hook
    sys.modules["antenv.axon_hooks"] = mod
    antenv.axon_hooks = mod

    so_path = "/opt/axon/libaxon_pjrt.so"
    try:
        sys.path.insert(0, "/root/.axon_site")
        from trn_agent_boot.trn_boot import _ntff_profile_via_ctypes
        hook = _ntff_profile_via_ctypes(so_path)
        if hook is not None:
            mod._hook = hook
    except Exception as e:  # degrade: tracing skipped, run still works
        print(f"ntff hook setup failed: {e}", file=sys.stderr)


def kernel(x, K_depth, V_depth, Wq, Wk, Wv, Wkd, Wvd, Wo, _trace=False):
    from concourse.bass_utils import run_bass_kernel_spmd

    if _trace:
        _ensure_ntff_hook()

    x = np.asarray(x, dtype=np.float32)
    K_depth = np.asarray(K_depth, dtype=np.float32)
    V_depth = np.asarray(V_depth, dtype=np.float32)
    Wq, Wk, Wv, Wkd, Wvd, Wo = (np.asarray(w, dtype=np.float32)
                                for w in (Wq, Wk, Wv, Wkd, Wvd, Wo))

    consts = _const_inputs()
    in_maps = []
    for core in range(8):
        b, h = divmod(core, HK)
        m = _make_core_inputs(x, K_depth, V_depth, Wq, Wk, Wv, Wkd, Wvd, Wo, b, h)
        m.update(consts)
        in_maps.append(m)

    nc = _get_nc()
    res = run_bass_kernel_spmd(nc, in_maps, list(range(8)), trace=_trace)
    kernel.last_result = res

    out = np.zeros((B, T, D), dtype=np.float32)
    k_write = np.zeros((B, HK, T, HD), dtype=np.float32)
    v_write = np.zeros((B, HK, T, HD), dtype=np.float32)
    for core in range(8):
        b, h = divmod(core, HK)
        out[b] += res.results[core]["y"]
        kvT = res.results[core]["kvT"]
        k_write[b, h] = kvT[0:64, :].T
        v_write[b, h] = kvT[64:128, :].T
    return out, k_write, v_write


# revision 22
# speedup vs baseline: 1.1917x; 1.1917x over previous
"""MoDA attention Trainium2 kernel.

nn_MoDAAttention: B=2, T=2048, D=1024, HQ=16, HK=4, HD=64, LP=8.

Sharding: 8 cores = (batch 2) x (kv-head 4). Each core handles one batch row
and one kv head (4 query heads, GQA group). Depth KV cache shards along HK.
o_proj weight row-sharded by head group; partial outputs summed on host.

Per-core plan (all on one NeuronCore, Tile-scheduled):
  phase 1: projections  QT/KT/VT/kdT/vdT = W.T @ xT   (fp32r matmuls)
  phase 2a: depth attention scores in transposed orientation:
            prod_l = QT_bf16 * KdT_l (DVE, bf16 2x), s_dep^T = ones-selector
            matmul over d (PE), exp (ACT) -> p_dep^T; transpose to p_nat;
            depth PV: p_nat (bcast) * Vd_dmaj -> reduce over l -> O_dep (nat).
  phase 2b: per q-chunk of 512: S^T = K^T.T @ Q^T (row-packed pairs of K=64
            matmuls), exp -> P^T bf16, PV: [V|1].T @ P^T accumulates
            [O^T; den] in PSUM over k-tiles; depth den + O_dep^T folded into
            the same PSUM via selector/transpose matmuls; reciprocal ->
            K=1 broadcast matmul -> H^T = O^T * (1/den).
  phase 3: o_proj: Y = H^T.T @ Wo (fp32r), DMA out.

kernel(**inputs) takes FULL inputs, shards + lays out on host (numpy),
runs SPMD on cores 0-7, gathers on host.
"""

import sys
from contextlib import ExitStack

import numpy as np

for _p in ("/opt/trn_rl_repo", "/opt/trn_rl_repo/concourse"):
    if _p not in sys.path:
        sys.path.insert(0, _p)

import ml_dtypes

import concourse.bass as bass
import concourse.mybir as mybir
import concourse.tile as tile
from concourse import bacc
from concourse.masks import make_identity

F32 = mybir.dt.float32
F32R = mybir.dt.float32r
BF16 = mybir.dt.float16
BF = np.float16

B, T, D, HQ, HK, HD, LP = 2, 2048, 1024, 16, 4, 64, 8
G = HQ // HK            # q heads per core = 4
NKI = T // 128          # 16 k-tiles
NQC = T // 512          # 4 q-chunks
NDC = D // 128          # 8 D chunks
SCALE = HD ** -0.5      # 1/8


DEBUG_OUTPUTS = False


def r(ap):
    return ap.bitcast(F32R)


def build_nc():
    nc = bacc.Bacc("TRN2", target_bir_lowering=False, debug=False)

    # ---- DRAM I/O ----
    xT = nc.dram_tensor("xT", [D, T], F32R, kind="ExternalInput")
    w_proj = nc.dram_tensor("w_proj", [NDC, 128, 512], F32R, kind="ExternalInput")
    wo = nc.dram_tensor("wo", [2, 128, D], F32R, kind="ExternalInput")
    kdT = nc.dram_tensor("kdT", [LP, 128, T], BF16, kind="ExternalInput")
    vd_dmaj = nc.dram_tensor("vd_dmaj", [T, LP * HD], BF16, kind="ExternalInput")
    trimask = nc.dram_tensor("trimask", [128, 128], BF16, kind="ExternalInput")
    sel = nc.dram_tensor("sel", [LP, 128, 16], BF16, kind="ExternalInput")
    densel = nc.dram_tensor("densel", [2, 16, 66], F32R, kind="ExternalInput")
    identity = nc.dram_tensor("identity", [128, 128], F32R, kind="ExternalInput")
    ones64 = nc.dram_tensor("ones64", [1, 64], F32R, kind="ExternalInput")

    y_out = nc.dram_tensor("y", [T, D], F32, kind="ExternalOutput")
    kvT_out = nc.dram_tensor("kvT", [128, T], F32, kind="ExternalOutput")
    if DEBUG_OUTPUTS:
        dbg_pdepT = nc.dram_tensor("dbg_pdepT", [16, T], F32, kind="ExternalOutput")
        dbg_odep = nc.dram_tensor("dbg_odep", [128, NKI * HD], F32,
                                  kind="ExternalOutput")
        dbg_HT = nc.dram_tensor("dbg_HT", [128, T], F32, kind="ExternalOutput")
        dbg_pt = nc.dram_tensor("dbg_pt", [128, 512], F32, kind="ExternalOutput")
        dbg_qt = nc.dram_tensor("dbg_qt", [128, T], F32, kind="ExternalOutput")
        dbg_pnat = nc.dram_tensor("dbg_pnat", [128, NKI * 16], F32,
                                  kind="ExternalOutput")
        dbg_pv1 = nc.dram_tensor("dbg_pv1", [66, 512], F32, kind="ExternalOutput")
        dbg_pv2 = nc.dram_tensor("dbg_pv2", [66, 512], F32, kind="ExternalOutput")
        dbg_bcs = nc.dram_tensor("dbg_bcs", [64, 512], F32, kind="ExternalOutput")
        dbg_rec = nc.dram_tensor("dbg_rec", [1, 512], F32, kind="ExternalOutput")
        dbg_den = nc.dram_tensor("dbg_den", [1, 512], F32, kind="ExternalOutput")

    with tile.TileContext(nc) as tc, ExitStack() as ctx:
        const = ctx.enter_context(tc.tile_pool(name="const", bufs=1))
        persist = ctx.enter_context(tc.tile_pool(name="persist", bufs=1))

        # constants
        w_sb = [const.tile([128, 512], F32R, name=f"w{dc}") for dc in range(NDC)]
        for dc in range(NDC):
            nc.sync.dma_start(w_sb[dc][:], w_proj[dc])
        wo_sb = [const.tile([128, D], F32R, name=f"wo{p}") for p in range(2)]
        for p in range(2):
            nc.sync.dma_start(wo_sb[p][:], wo[p])
        trimask_sb = const.tile([128, 128], BF16)
        nc.sync.dma_start(trimask_sb[:], trimask[:])
        sel_sb = [const.tile([128, 16], BF16, name=f"sel{l}") for l in range(LP)]
        for l in range(LP):
            nc.sync.dma_start(sel_sb[l][:], sel[l])
        densel_sb = [const.tile([16, 66], F32R, name=f"densel{hp}") for hp in range(2)]
        for hp in range(2):
            nc.sync.dma_start(densel_sb[hp][:], densel[hp])
        ones64_sb = const.tile([1, 64], F32R)
        nc.sync.dma_start(ones64_sb[:], ones64[:])
        ident = const.tile([128, 128], F32R)
        nc.sync.dma_start(ident[:], identity[:])

        # persistent intermediates
        QTbf = [persist.tile([128, T], BF16, name=f"QTbf{p}") for p in range(2)]
        vk_sb = persist.tile([128, T], F32R, name="vk")      # rows 0:64 V^T, 64:128 K^T
        kv_sb = persist.tile([128, T], F32, name="kv")      # rows 0:64 kd^T, 64:128 vd^T
        kt16 = persist.tile([128, T], BF16, name="kt16")     # K^T fp16, duplicated halves
        vones = [persist.tile([128, 66], BF16, name=f"vones{ki}") for ki in range(NKI)]
        pdepT = [persist.tile([16, T], F32R, name=f"pdepT{p}") for p in range(2)]
        p_nat = [persist.tile([128, NKI * 16], BF16, name=f"pnat{p}") for p in range(2)]
        odep = [persist.tile([128, NKI * HD], F32R, name=f"odep{h}") for h in range(G)]
        HT = [persist.tile([128, T], F32R, name=f"HT{p}") for p in range(2)]

        # ---------------- phase 1: projections ----------------
        xTv = xT[:].rearrange("(c p) t -> p c t", p=128)
        with tc.tile_pool(name="xpool", bufs=2) as xpool, \
             tc.tile_pool(name="ppsum", bufs=2, space="PSUM") as ppsum:
            for tci in range(NQC):
                psums = [ppsum.tile([128, 512], F32, name=f"pj{m}") for m in range(4)]
                xt = xpool.tile([128, NDC, 512], F32R)
                nc.sync.dma_start(
                    xt[:], xTv[:, :, tci * 512:(tci + 1) * 512])
                for dc in range(NDC):
                    for m in range(4):
                        nc.tensor.matmul(
                            psums[m][:], w_sb[dc][:, m * 128:(m + 1) * 128],
                            xt[:, dc, :], start=(dc == 0), stop=(dc == NDC - 1))
                ts = slice(tci * 512, (tci + 1) * 512)
                nc.scalar.copy(QTbf[0][:, ts], psums[0][:])
                nc.scalar.copy(QTbf[1][:, ts], psums[1][:])
                nc.vector.tensor_copy(vk_sb[:, ts], psums[2][:])
                nc.vector.tensor_copy(kv_sb[:, ts], psums[3][:])

        # kd/vd out; K^T to fp16 + dup to low partitions
        nc.gpsimd.dma_start(kvT_out[:], kv_sb[:])
        nc.vector.tensor_copy(kt16[64:128, :], vk_sb[64:128, :])
        nc.sync.dma_start(kt16[0:64, :], kt16[64:128, :])

        # [V|1] tiles: transpose V^T chunks, cast to bf16, ones column
        with tc.tile_pool(name="vtp", bufs=4, space="PSUM") as vtp:
            for ki in range(NKI):
                tp = vtp.tile([128, 64], F32)
                nc.tensor.matmul(r(tp[:]), vk_sb[0:64, ki * 128:(ki + 1) * 128],
                                 ident[0:64, 0:64], is_transpose=True, start=True,
                                 stop=True)
                nc.scalar.copy(vones[ki][:, 0:64], tp[:])
                nc.vector.memset(vones[ki][:, 64:65], 1.0)
                nc.vector.memset(vones[ki][:, 65:66], 0.0)

        # ------- phase 2: depth + sequence attention, interleaved per qc -------
        with tc.tile_pool(name="kdpool", bufs=8) as kdpool, \
             tc.tile_pool(name="dprod", bufs=4) as dprod, \
             tc.tile_pool(name="vdpool", bufs=4) as vdpool, \
             tc.tile_pool(name="dvprod", bufs=4) as dvprod, \
             tc.tile_pool(name="ptpool", bufs=6) as ptpool, \
             tc.tile_pool(name="small", bufs=2) as small, \
             tc.tile_pool(name="dpsum", bufs=1, space="PSUM") as dpsum, \
             tc.tile_pool(name="stps", bufs=4, space="PSUM") as stps, \
             tc.tile_pool(name="pvps", bufs=1, space="PSUM") as pvps, \
             tc.tile_pool(name="bcps", bufs=1, space="PSUM") as bcps:
            kdt_sb = [kdpool.tile([128, T], BF16, name="kdt") for _ in range(LP)]
            for l in range(LP):
                nc.sync.dma_start(kdt_sb[l][:], kdT[l])
            for qci in range(NQC):
                qs = slice(qci * 512, (qci + 1) * 512)
                # depth scores (both pairs) for this q-chunk
                for pr in range(2):
                    sdep = dpsum.tile([16, 512], F32, name="dp")
                    for l in range(LP):
                        prod = dprod.tile([128, 512], BF16, name="prod")
                        nc.vector.tensor_tensor(
                            prod[:], QTbf[pr][:, qs], kdt_sb[l][:, qs],
                            op=mybir.AluOpType.mult)
                        nc.tensor.matmul(sdep[:], sel_sb[l][:], prod[:],
                                         start=(l == 0), stop=(l == LP - 1))
                    nc.scalar.activation(pdepT[pr][:, qs], sdep[:],
                                         mybir.ActivationFunctionType.Exp,
                                         scale=SCALE)
                    pn = dpsum.tile([128, 64], F32, name="dp")
                    for j in range(4):
                        cs = slice(qci * 512 + j * 128, qci * 512 + (j + 1) * 128)
                        nc.tensor.matmul(
                            r(pn[:, j * 16:(j + 1) * 16]), pdepT[pr][:, cs],
                            ident[0:16, 0:16], is_transpose=True, start=True,
                            stop=True, skip_group_check=True)
                    nc.vector.tensor_copy(
                        p_nat[pr][:, qci * 64:(qci + 1) * 64], pn[:])
                # depth PV for this q-chunk's t-tiles
                for j in range(4):
                    tt = qci * 4 + j
                    vd = vdpool.tile([128, LP * HD], BF16, name="vd")
                    nc.gpsimd.dma_start(vd[:], vd_dmaj[tt * 128:(tt + 1) * 128, :])
                    vdv = vd[:].rearrange("p (d l) -> p d l", l=LP)
                    for h in range(G):
                        pr, hp = divmod(h, 2)
                        base = tt * 16 + hp * 8
                        pb = (p_nat[pr][:, base:base + 8]
                              .rearrange("p (o l) -> p o l", o=1)
                              .to_broadcast((128, HD, LP)))
                        pvp = dvprod.tile([128, LP * HD], BF16, name="pv")
                        pvv = pvp[:].rearrange("p (d l) -> p d l", l=LP)
                        nc.vector.tensor_tensor(pvv, pb, vdv,
                                                op=mybir.AluOpType.mult)
                        with nc.allow_low_precision(reason="f32r out, fp32 acc"):
                            nc.vector.tensor_reduce(
                                odep[h][:, tt * HD:(tt + 1) * HD], pvv,
                                axis=mybir.AxisListType.X, op=mybir.AluOpType.add)
                # sequence attention, one pass per head-pair (2 psum banks each)
                n_ki = 4 * qci + 4
                for pr in range(2):
                    pv = [pvps.tile([66, 512], F32, name=f"pv{hp}")
                          for hp in range(2)]
                    pts = [None, None]
                    # software-pipelined: emit S^T(ki) before PV(ki-1) so the
                    # PE stream doesn't stall on the exp (ACT) dependency
                    for ki in range(n_ki + 1):
                        if ki < n_ki:
                            ks = slice(ki * 128, (ki + 1) * 128)
                            st0 = stps.tile([128, 512], F32, name="st")
                            st1 = stps.tile([128, 512], F32, name="st")
                            nc.tensor.matmul(st0[:], kt16[0:64, ks],
                                             QTbf[pr][0:64, qs], start=True,
                                             stop=True, tile_position=(0, 0))
                            nc.tensor.matmul(st1[:], kt16[64:128, ks],
                                             QTbf[pr][64:128, qs], start=True,
                                             stop=True, tile_position=(64, 0))
                            newpts = []
                            for hp, st in ((0, st0), (1, st1)):
                                pt = ptpool.tile([128, 512], BF16, name="pt")
                                left = ki * 128 - qci * 512
                                if left >= 0:
                                    if left > 0:
                                        nc.vector.memset(pt[:, 0:left], 0.0)
                                    nc.scalar.activation(
                                        pt[:, left:512], st[:, left:512],
                                        mybir.ActivationFunctionType.Exp,
                                        scale=SCALE)
                                    nc.vector.tensor_tensor(
                                        pt[:, left:left + 128],
                                        pt[:, left:left + 128],
                                        trimask_sb[:], op=mybir.AluOpType.mult)
                                else:
                                    nc.scalar.activation(
                                        pt[:], st[:],
                                        mybir.ActivationFunctionType.Exp,
                                        scale=SCALE)
                                newpts.append(pt)
                        if ki >= 1:
                            for hp in range(2):
                                nc.tensor.matmul(
                                    pv[hp][0:66, :], vones[ki - 1][:],
                                    pts[hp][:], start=(ki == 1), stop=False,
                                    skip_group_check=True)
                        if ki < n_ki:
                            pts = newpts
                    # fold depth den + O_dep^T, then normalize
                    for hp in range(2):
                        h = 2 * pr + hp
                        nc.tensor.matmul(pv[hp][0:66, :], densel_sb[hp][:],
                                         pdepT[pr][:, qs], start=False,
                                         stop=False, skip_group_check=True)
                        for j in range(4):
                            tt = qci * 4 + j
                            nc.tensor.matmul(
                                r(pv[hp][0:64, j * 128:(j + 1) * 128]),
                                odep[h][:, tt * HD:(tt + 1) * HD], ident[:],
                                is_transpose=True, start=False, stop=(j == 3),
                                skip_group_check=True)
                    for hp in range(2):
                        h = 2 * pr + hp
                        rec = small.tile([1, 512], F32, name="rec")
                        scr = small.tile([1, 512], F32, name="scr")
                        dsb = small.tile([65, 512], F32, name="dsb")
                        dlow = small.tile([1, 512], F32, name="dlow")
                        # den on partition 64; engines can't cross partitions
                        # and the custom recip needs base 0: DVE copy out of
                        # psum on partition 64, DMA down to partition 0.
                        nc.vector.tensor_copy(dsb[64:65, :], pv[hp][64:65, :])
                        nc.sync.dma_start(dlow[:], dsb[64:65, :])
                        nc.vector.reciprocal_approx_accurate(
                            rec[:], dlow[:], scr[:])
                        rec_r = small.tile([1, 512], F32R, name="recr")
                        nc.vector.tensor_copy(rec_r[:], rec[:])
                        bc = bcps.tile([64, 512], F32, name="bc")
                        nc.tensor.matmul(bc[:], ones64_sb[:], rec_r[:],
                                         start=True, stop=True)
                        bcs = small.tile([64, 512], F32, name="bcs")
                        nc.vector.tensor_copy(bcs[:], bc[:])
                        if hp == 0:
                            nc.vector.tensor_tensor(
                                HT[pr][0:64, qs], pv[hp][0:64, :],
                                bcs[:], op=mybir.AluOpType.mult)
                        else:
                            httmp = small.tile([64, 512], F32R, name="httmp")
                            nc.vector.tensor_tensor(
                                httmp[:], pv[hp][0:64, :], bcs[:],
                                op=mybir.AluOpType.mult)
                            nc.sync.dma_start(HT[pr][64:128, qs], httmp[:])

        if DEBUG_OUTPUTS:
            dbg1 = persist.tile([16, T], F32, name="dbg1")
            nc.vector.tensor_copy(dbg1[:], pdepT[0][:])
            nc.sync.dma_start(dbg_pdepT[:], dbg1[:])
            dbg2 = persist.tile([128, NKI * HD], F32, name="dbg2")
            nc.vector.tensor_copy(dbg2[:], odep[0][:])
            nc.sync.dma_start(dbg_odep[:], dbg2[:])
            dbg3 = persist.tile([128, T], F32, name="dbg3")
            nc.vector.tensor_copy(dbg3[:], HT[0][:])
            nc.sync.dma_start(dbg_HT[:], dbg3[:])
            dbg4 = persist.tile([128, T], F32, name="dbg4")
            nc.vector.tensor_copy(dbg4[:], QTbf[0][:])
            nc.sync.dma_start(dbg_qt[:], dbg4[:])
            dbg5 = persist.tile([128, NKI * 16], F32, name="dbg5")
            nc.vector.tensor_copy(dbg5[:], p_nat[0][:])
            nc.sync.dma_start(dbg_pnat[:], dbg5[:])

        # ---------------- phase 3: o_proj ----------------
        with tc.tile_pool(name="ypsum", bufs=4, space="PSUM") as ypsum, \
             tc.tile_pool(name="ypool", bufs=3) as ypool:
            for tt in range(NKI):
                cs = slice(tt * 128, (tt + 1) * 128)
                ysb = ypool.tile([128, D], F32, name="ysb")
                for ec in range(2):
                    es = slice(ec * 512, (ec + 1) * 512)
                    yp = ypsum.tile([128, 512], F32, name="yp")
                    for p in range(2):
                        nc.tensor.matmul(yp[:], HT[p][:, cs], wo_sb[p][:, es],
                                         start=(p == 0), stop=(p == 1))
                    if ec == 0:
                        nc.scalar.copy(ysb[:, es], yp[:])
                    else:
                        nc.vector.tensor_copy(ysb[:, es], yp[:])
                nc.gpsimd.dma_start(y_out[cs, :], ysb[:])

    nc.compile()
    return nc


_NC_CACHE = None


def _get_nc():
    global _NC_CACHE
    if _NC_CACHE is None:
        _NC_CACHE = build_nc()
    return _NC_CACHE


def _make_core_inputs(x, K_depth, V_depth, Wq, Wk, Wv, Wkd, Wvd, Wo, b, h):
    xT = np.ascontiguousarray(x[b].T)                       # (D, T)
    wq = Wq[:, h * G * HD:(h + 1) * G * HD]                 # (D, 256)
    wk = Wk[:, h * HD:(h + 1) * HD]
    wv = Wv[:, h * HD:(h + 1) * HD]
    wkd = Wkd[:, h * HD:(h + 1) * HD]
    wvd = Wvd[:, h * HD:(h + 1) * HD]
    # m0=[q0|q1] m1=[q2|q3] m2=[V|K] m3=[kd|vd]
    w_stack = np.concatenate([wq, wv, wk, wkd, wvd], axis=1)  # (D, 512)
    w_proj = np.ascontiguousarray(w_stack.reshape(NDC, 128, 512), dtype=np.float32)
    wo = np.ascontiguousarray(
        Wo[h * G * HD:(h + 1) * G * HD, :].reshape(2, 128, D), dtype=np.float32)
    kd = K_depth[b, h].reshape(T, LP, HD)
    kdT_l = np.transpose(kd, (1, 2, 0))                     # (LP, HD, T)
    kdT = np.concatenate([kdT_l, kdT_l], axis=1).astype(BF)  # (LP, 128, T)
    vd = V_depth[b, h].reshape(T, LP, HD)
    vd_dmaj = np.ascontiguousarray(
        np.transpose(vd, (0, 2, 1)).reshape(T, LP * HD)).astype(BF)
    return dict(xT=xT, w_proj=w_proj, wo=wo, kdT=np.ascontiguousarray(kdT),
                vd_dmaj=vd_dmaj)


def _const_inputs():
    k = np.arange(128)[:, None]
    q = np.arange(128)[None, :]
    trimask = (q >= k).astype(BF)                            # (128, 128)
    sel = np.zeros((LP, 128, 16), dtype=BF)
    for l in range(LP):
        sel[l, 0:64, l] = 1.0
        sel[l, 64:128, 8 + l] = 1.0
    densel = np.zeros((2, 16, 66), dtype=np.float32)
    for hp in range(2):
        densel[hp, hp * 8:(hp + 1) * 8, 64] = 1.0
    ones64 = np.ones((1, 64), dtype=np.float32)
    identity = np.eye(128, dtype=np.float32)
    return dict(trimask=trimask, sel=sel, densel=densel, ones64=ones64,
                identity=identity)


def _ensure_ntff_hook():
    """Provide the antenv.axon_hooks shim (missing in this image) so
    run_bass_kernel_spmd(trace=True) can capture NTFF profiles via the
    ctypes hook in libaxon_pjrt.so (mirrors trn_agent_boot.trn_boot)."""
    import types

    try:
        from antenv.axon_hooks import get_axon_ntff_profile_hook  # noqa: F401
        return
    except ImportError:
        pass
    import antenv

    mod = types.ModuleType("antenv.axon_hooks")
    mod._hook = None

    def set_axon_ntff_profile_hook(h):
        mod._hook = h

    def get_axon_ntff_profile_hook():
        return mod._hook

    mod.set_axon_ntff_profile_hook = set_axon_ntff_profile_hook
    mod.get_axon_ntff_profile_hook = get_axon_ntff_profile_hook
    sys.modules["antenv.axon_hooks"] = mod
    antenv.axon_hooks = mod

    so_path = "/opt/axon/libaxon_pjrt.so"
    try:
        sys.path.insert(0, "/root/.axon_site")
        from trn_agent_boot.trn_boot import _ntff_profile_via_ctypes
        hook = _ntff_profile_via_ctypes(so_path)
        if hook is not None:
            mod._hook = hook
    except Exception as e:  # degrade: tracing skipped, run still works
        print(f"ntff hook setup failed: {e}", file=sys.stderr)


def kernel(x, K_depth, V_depth, Wq, Wk, Wv, Wkd, Wvd, Wo, _trace=False):
    from concourse.bass_utils import run_bass_kernel_spmd

    if _trace:
        _ensure_ntff_hook()

    x = np.asarray(x, dtype=np.float32)
    K_depth = np.asarray(K_depth, dtype=np.float32)
    V_depth = np.asarray(V_depth, dtype=np.float32)
    Wq, Wk, Wv, Wkd, Wvd, Wo = (np.asarray(w, dtype=np.float32)
                                for w in (Wq, Wk, Wv, Wkd, Wvd, Wo))

    consts = _const_inputs()
    in_maps = []
    for core in range(8):
        b, h = divmod(core, HK)
        m = _make_core_inputs(x, K_depth, V_depth, Wq, Wk, Wv, Wkd, Wvd, Wo, b, h)
        m.update(consts)
        in_maps.append(m)

    nc = _get_nc()
    res = run_bass_kernel_spmd(nc, in_maps, list(range(8)), trace=_trace)
    kernel.last_result = res

    out = np.zeros((B, T, D), dtype=np.float32)
    k_write = np.zeros((B, HK, T, HD), dtype=np.float32)
    v_write = np.zeros((B, HK, T, HD), dtype=np.float32)
    for core in range(8):
        b, h = divmod(core, HK)
        out[b] += res.results[core]["y"]
        kvT = res.results[core]["kvT"]
        k_write[b, h] = kvT[0:64, :].T
        v_write[b, h] = kvT[64:128, :].T
    return out, k_write, v_write


# revision 24
# speedup vs baseline: 1.3065x; 1.0964x over previous
"""MoDA attention Trainium2 kernel.

nn_MoDAAttention: B=2, T=2048, D=1024, HQ=16, HK=4, HD=64, LP=8.

Sharding: 8 cores = (batch 2) x (kv-head 4). Each core handles one batch row
and one kv head (4 query heads, GQA group). Depth KV cache shards along HK.
o_proj weight row-sharded by head group; partial outputs summed on host.

Per-core plan (all on one NeuronCore, Tile-scheduled):
  phase 1: projections  QT/KT/VT/kdT/vdT = W.T @ xT   (fp32r matmuls)
  phase 2a: depth attention scores in transposed orientation:
            prod_l = QT_bf16 * KdT_l (DVE, bf16 2x), s_dep^T = ones-selector
            matmul over d (PE), exp (ACT) -> p_dep^T; transpose to p_nat;
            depth PV: p_nat (bcast) * Vd_dmaj -> reduce over l -> O_dep (nat).
  phase 2b: per q-chunk of 512: S^T = K^T.T @ Q^T (row-packed pairs of K=64
            matmuls), exp -> P^T bf16, PV: [V|1].T @ P^T accumulates
            [O^T; den] in PSUM over k-tiles; depth den + O_dep^T folded into
            the same PSUM via selector/transpose matmuls; reciprocal ->
            K=1 broadcast matmul -> H^T = O^T * (1/den).
  phase 3: o_proj: Y = H^T.T @ Wo (fp32r), DMA out.

kernel(**inputs) takes FULL inputs, shards + lays out on host (numpy),
runs SPMD on cores 0-7, gathers on host.
"""

import sys
from contextlib import ExitStack

import numpy as np

for _p in ("/opt/trn_rl_repo", "/opt/trn_rl_repo/concourse"):
    if _p not in sys.path:
        sys.path.insert(0, _p)

import ml_dtypes

import concourse.bass as bass
import concourse.mybir as mybir
import concourse.tile as tile
from concourse import bacc
from concourse.masks import make_identity

F32 = mybir.dt.float32
F32R = mybir.dt.float32r
BF16 = mybir.dt.float16
BF = np.float16

B, T, D, HQ, HK, HD, LP = 2, 2048, 1024, 16, 4, 64, 8
G = HQ // HK            # q heads per core = 4
NKI = T // 128          # 16 k-tiles
NQC = T // 512          # 4 q-chunks
NDC = D // 128          # 8 D chunks
SCALE = HD ** -0.5      # 1/8


DEBUG_OUTPUTS = False


def r(ap):
    return ap.bitcast(F32R)


def build_nc():
    nc = bacc.Bacc("TRN2", target_bir_lowering=False, debug=False)

    # ---- DRAM I/O ----
    xT = nc.dram_tensor("xT", [D, T], F32R, kind="ExternalInput")
    w_proj = nc.dram_tensor("w_proj", [NDC, 128, 512], F32R, kind="ExternalInput")
    wo = nc.dram_tensor("wo", [2, 128, D], BF16, kind="ExternalInput")
    kdT = nc.dram_tensor("kdT", [LP, 128, T], BF16, kind="ExternalInput")
    vd_dmaj = nc.dram_tensor("vd_dmaj", [T, LP * HD], BF16, kind="ExternalInput")
    trimask = nc.dram_tensor("trimask", [128, 128], BF16, kind="ExternalInput")
    sel = nc.dram_tensor("sel", [LP, 128, 16], BF16, kind="ExternalInput")
    densel = nc.dram_tensor("densel", [2, 16, 66], F32R, kind="ExternalInput")
    identity = nc.dram_tensor("identity", [128, 128], F32R, kind="ExternalInput")
    ones64 = nc.dram_tensor("ones64", [1, 64], F32R, kind="ExternalInput")

    y_out = nc.dram_tensor("y", [T, D], F32, kind="ExternalOutput")
    kvT_out = nc.dram_tensor("kvT", [128, T], F32, kind="ExternalOutput")
    if DEBUG_OUTPUTS:
        dbg_pdepT = nc.dram_tensor("dbg_pdepT", [16, T], F32, kind="ExternalOutput")
        dbg_odep = nc.dram_tensor("dbg_odep", [128, NKI * HD], F32,
                                  kind="ExternalOutput")
        dbg_HT = nc.dram_tensor("dbg_HT", [128, T], F32, kind="ExternalOutput")
        dbg_pt = nc.dram_tensor("dbg_pt", [128, 512], F32, kind="ExternalOutput")
        dbg_qt = nc.dram_tensor("dbg_qt", [128, T], F32, kind="ExternalOutput")
        dbg_pnat = nc.dram_tensor("dbg_pnat", [128, NKI * 16], F32,
                                  kind="ExternalOutput")
        dbg_pv1 = nc.dram_tensor("dbg_pv1", [66, 512], F32, kind="ExternalOutput")
        dbg_pv2 = nc.dram_tensor("dbg_pv2", [66, 512], F32, kind="ExternalOutput")
        dbg_bcs = nc.dram_tensor("dbg_bcs", [64, 512], F32, kind="ExternalOutput")
        dbg_rec = nc.dram_tensor("dbg_rec", [1, 512], F32, kind="ExternalOutput")
        dbg_den = nc.dram_tensor("dbg_den", [1, 512], F32, kind="ExternalOutput")

    with tile.TileContext(nc) as tc, ExitStack() as ctx:
        const = ctx.enter_context(tc.tile_pool(name="const", bufs=1))
        persist = ctx.enter_context(tc.tile_pool(name="persist", bufs=1))

        # constants
        w_sb = [const.tile([128, 512], F32R, name=f"w{dc}") for dc in range(NDC)]
        for dc in range(NDC):
            nc.sync.dma_start(w_sb[dc][:], w_proj[dc])
        wo_sb = [const.tile([128, D], BF16, name=f"wo{p}") for p in range(2)]
        for p in range(2):
            nc.sync.dma_start(wo_sb[p][:], wo[p])
        trimask_sb = const.tile([128, 128], BF16)
        nc.sync.dma_start(trimask_sb[:], trimask[:])
        sel_sb = [const.tile([128, 16], BF16, name=f"sel{l}") for l in range(LP)]
        for l in range(LP):
            nc.sync.dma_start(sel_sb[l][:], sel[l])
        densel_sb = [const.tile([16, 66], F32R, name=f"densel{hp}") for hp in range(2)]
        for hp in range(2):
            nc.sync.dma_start(densel_sb[hp][:], densel[hp])
        ones64_sb = const.tile([1, 64], F32R)
        nc.sync.dma_start(ones64_sb[:], ones64[:])
        ident = const.tile([128, 128], F32R)
        nc.sync.dma_start(ident[:], identity[:])

        # persistent intermediates
        QTbf = [persist.tile([128, T], BF16, name=f"QTbf{p}") for p in range(2)]
        vk_sb = persist.tile([128, T], F32R, name="vk")      # rows 0:64 V^T, 64:128 K^T
        kv_sb = persist.tile([128, T], F32, name="kv")      # rows 0:64 kd^T, 64:128 vd^T
        kt16 = persist.tile([128, T], BF16, name="kt16")     # K^T fp16, duplicated halves
        vones = [persist.tile([128, 66], BF16, name=f"vones{ki}") for ki in range(NKI)]
        pdepT = [persist.tile([16, T], F32R, name=f"pdepT{p}") for p in range(2)]
        p_nat = [persist.tile([128, NKI * 16], BF16, name=f"pnat{p}") for p in range(2)]
        odep = [persist.tile([128, NKI * HD], F32R, name=f"odep{h}") for h in range(G)]
        HT = [persist.tile([128, T], BF16, name=f"HT{p}") for p in range(2)]

        # ---------------- phase 1: projections ----------------
        xTv = xT[:].rearrange("(c p) t -> p c t", p=128)
        with tc.tile_pool(name="xpool", bufs=2) as xpool, \
             tc.tile_pool(name="ppsum", bufs=2, space="PSUM") as ppsum:
            for tci in range(NQC):
                psums = [ppsum.tile([128, 512], F32, name=f"pj{m}") for m in range(4)]
                xt = xpool.tile([128, NDC, 512], F32R)
                nc.sync.dma_start(
                    xt[:], xTv[:, :, tci * 512:(tci + 1) * 512])
                for dc in range(NDC):
                    for m in range(4):
                        nc.tensor.matmul(
                            psums[m][:], w_sb[dc][:, m * 128:(m + 1) * 128],
                            xt[:, dc, :], start=(dc == 0), stop=(dc == NDC - 1))
                ts = slice(tci * 512, (tci + 1) * 512)
                nc.scalar.copy(QTbf[0][:, ts], psums[0][:])
                nc.scalar.copy(QTbf[1][:, ts], psums[1][:])
                nc.vector.tensor_copy(vk_sb[:, ts], psums[2][:])
                nc.vector.tensor_copy(kv_sb[:, ts], psums[3][:])

        # kd/vd out; K^T to fp16 + dup to low partitions
        nc.sync.dma_start(kvT_out[:], kv_sb[:])
        nc.vector.tensor_copy(kt16[64:128, :], vk_sb[64:128, :])
        nc.sync.dma_start(kt16[0:64, :], kt16[64:128, :])

        # [V|1] tiles: transpose V^T chunks, cast to bf16, ones column
        with tc.tile_pool(name="vtp", bufs=4, space="PSUM") as vtp:
            for ki in range(NKI):
                tp = vtp.tile([128, 64], F32)
                nc.tensor.matmul(r(tp[:]), vk_sb[0:64, ki * 128:(ki + 1) * 128],
                                 ident[0:64, 0:64], is_transpose=True, start=True,
                                 stop=True)
                nc.scalar.copy(vones[ki][:, 0:64], tp[:])
                nc.vector.memset(vones[ki][:, 64:65], 1.0)
                nc.vector.memset(vones[ki][:, 65:66], 0.0)

        # ------- phase 2: depth + sequence attention, interleaved per qc -------
        with tc.tile_pool(name="kdpool", bufs=8) as kdpool, \
             tc.tile_pool(name="dprod", bufs=4) as dprod, \
             tc.tile_pool(name="vdpool", bufs=4) as vdpool, \
             tc.tile_pool(name="dvprod", bufs=4) as dvprod, \
             tc.tile_pool(name="ptpool", bufs=3) as ptpool, \
             tc.tile_pool(name="small", bufs=2) as small, \
             tc.tile_pool(name="dpsum", bufs=1, space="PSUM") as dpsum, \
             tc.tile_pool(name="stps", bufs=2, space="PSUM") as stps, \
             tc.tile_pool(name="pvps", bufs=1, space="PSUM") as pvps, \
             tc.tile_pool(name="bcps", bufs=1, space="PSUM") as bcps:
            kdt_sb = [kdpool.tile([128, T], BF16, name="kdt") for _ in range(LP)]
            for l in range(LP):
                nc.sync.dma_start(kdt_sb[l][:], kdT[l])
            for qci in range(NQC):
                qs = slice(qci * 512, (qci + 1) * 512)
                # depth scores (both pairs) for this q-chunk
                for pr in range(2):
                    sdep = dpsum.tile([16, 512], F32, name="dp")
                    for l in range(LP):
                        prod = dprod.tile([128, 512], BF16, name="prod")
                        nc.vector.tensor_tensor(
                            prod[:], QTbf[pr][:, qs], kdt_sb[l][:, qs],
                            op=mybir.AluOpType.mult)
                        nc.tensor.matmul(sdep[:], sel_sb[l][:], prod[:],
                                         start=(l == 0), stop=(l == LP - 1))
                    nc.scalar.activation(pdepT[pr][:, qs], sdep[:],
                                         mybir.ActivationFunctionType.Exp,
                                         scale=SCALE)
                    pn = dpsum.tile([128, 64], F32, name="dp")
                    for j in range(4):
                        cs = slice(qci * 512 + j * 128, qci * 512 + (j + 1) * 128)
                        nc.tensor.matmul(
                            r(pn[:, j * 16:(j + 1) * 16]), pdepT[pr][:, cs],
                            ident[0:16, 0:16], is_transpose=True, start=True,
                            stop=True, skip_group_check=True)
                    nc.vector.tensor_copy(
                        p_nat[pr][:, qci * 64:(qci + 1) * 64], pn[:])
                # depth PV for this q-chunk's t-tiles
                for j in range(4):
                    tt = qci * 4 + j
                    vd = vdpool.tile([128, LP * HD], BF16, name="vd")
                    nc.sync.dma_start(vd[:], vd_dmaj[tt * 128:(tt + 1) * 128, :])
                    vdv = vd[:].rearrange("p (d l) -> p d l", l=LP)
                    for h in range(G):
                        pr, hp = divmod(h, 2)
                        base = tt * 16 + hp * 8
                        pb = (p_nat[pr][:, base:base + 8]
                              .rearrange("p (o l) -> p o l", o=1)
                              .to_broadcast((128, HD, LP)))
                        pvp = dvprod.tile([128, LP * HD], BF16, name="pv")
                        pvv = pvp[:].rearrange("p (d l) -> p d l", l=LP)
                        nc.vector.tensor_tensor(pvv, pb, vdv,
                                                op=mybir.AluOpType.mult)
                        with nc.allow_low_precision(reason="f32r out, fp32 acc"):
                            nc.vector.tensor_reduce(
                                odep[h][:, tt * HD:(tt + 1) * HD], pvv,
                                axis=mybir.AxisListType.X, op=mybir.AluOpType.add)
                # sequence attention, one pass per head-pair (2 psum banks each)
                n_ki = 4 * qci + 4
                for pr in range(2):
                    pv = [pvps.tile([66, 512], F32, name=f"pv{hp}")
                          for hp in range(2)]
                    pts = [None, None]
                    # software-pipelined: emit S^T(ki) before PV(ki-1) so the
                    # PE stream doesn't stall on the exp (ACT) dependency
                    for ki in range(n_ki + 1):
                        if ki < n_ki:
                            ks = slice(ki * 128, (ki + 1) * 128)
                            st = stps.tile([128, 1024], F32, name="st")
                            nc.tensor.matmul(st[:, 0:512], kt16[0:64, ks],
                                             QTbf[pr][0:64, qs], start=True,
                                             stop=True, tile_position=(0, 0))
                            nc.tensor.matmul(st[:, 512:1024], kt16[64:128, ks],
                                             QTbf[pr][64:128, qs], start=True,
                                             stop=True, tile_position=(64, 0))
                            # one wide exp for both heads; masked-left region
                            # holds finite (masked) scores, zeroed after
                            pt = ptpool.tile([128, 1024], BF16, name="pt")
                            nc.scalar.activation(
                                pt[:], st[:], mybir.ActivationFunctionType.Exp,
                                scale=SCALE)
                            left = ki * 128 - qci * 512
                            if left >= 0:
                                for hp in range(2):
                                    o = hp * 512
                                    if left > 0:
                                        nc.vector.memset(pt[:, o:o + left], 0.0)
                                    nc.vector.tensor_tensor(
                                        pt[:, o + left:o + left + 128],
                                        pt[:, o + left:o + left + 128],
                                        trimask_sb[:], op=mybir.AluOpType.mult)
                        if ki >= 1:
                            for hp in range(2):
                                nc.tensor.matmul(
                                    pv[hp][0:66, :], vones[ki - 1][:],
                                    pts[hp], start=(ki == 1), stop=False,
                                    skip_group_check=True)
                        if ki < n_ki:
                            pts = [pt[:, 0:512], pt[:, 512:1024]]
                    # fold depth den + O_dep^T, then normalize
                    for hp in range(2):
                        h = 2 * pr + hp
                        nc.tensor.matmul(pv[hp][0:66, :], densel_sb[hp][:],
                                         pdepT[pr][:, qs], start=False,
                                         stop=False, skip_group_check=True)
                        for j in range(4):
                            tt = qci * 4 + j
                            nc.tensor.matmul(
                                r(pv[hp][0:64, j * 128:(j + 1) * 128]),
                                odep[h][:, tt * HD:(tt + 1) * HD], ident[:],
                                is_transpose=True, start=False, stop=(j == 3),
                                skip_group_check=True)
                    for hp in range(2):
                        h = 2 * pr + hp
                        rec = small.tile([1, 512], F32, name="rec")
                        scr = small.tile([1, 512], F32, name="scr")
                        dsb = small.tile([65, 512], F32, name="dsb")
                        dlow = small.tile([1, 512], F32, name="dlow")
                        # den on partition 64; engines can't cross partitions
                        # and the custom recip needs base 0: DVE copy out of
                        # psum on partition 64, DMA down to partition 0.
                        nc.vector.tensor_copy(dsb[64:65, :], pv[hp][64:65, :])
                        nc.sync.dma_start(dlow[:], dsb[64:65, :])
                        nc.vector.reciprocal_approx_accurate(
                            rec[:], dlow[:], scr[:])
                        rec_r = small.tile([1, 512], F32R, name="recr")
                        nc.vector.tensor_copy(rec_r[:], rec[:])
                        bc = bcps.tile([64, 512], F32, name="bc")
                        nc.tensor.matmul(bc[:], ones64_sb[:], rec_r[:],
                                         start=True, stop=True)
                        bcs = small.tile([64, 512], F32, name="bcs")
                        nc.vector.tensor_copy(bcs[:], bc[:])
                        if hp == 0:
                            nc.vector.tensor_tensor(
                                HT[pr][0:64, qs], pv[hp][0:64, :],
                                bcs[:], op=mybir.AluOpType.mult)
                        else:
                            httmp = small.tile([64, 512], BF16, name="httmp")
                            nc.vector.tensor_tensor(
                                httmp[:], pv[hp][0:64, :], bcs[:],
                                op=mybir.AluOpType.mult)
                            nc.sync.dma_start(HT[pr][64:128, qs], httmp[:])

        if DEBUG_OUTPUTS:
            dbg1 = persist.tile([16, T], F32, name="dbg1")
            nc.vector.tensor_copy(dbg1[:], pdepT[0][:])
            nc.sync.dma_start(dbg_pdepT[:], dbg1[:])
            dbg2 = persist.tile([128, NKI * HD], F32, name="dbg2")
            nc.vector.tensor_copy(dbg2[:], odep[0][:])
            nc.sync.dma_start(dbg_odep[:], dbg2[:])
            dbg3 = persist.tile([128, T], F32, name="dbg3")
            nc.vector.tensor_copy(dbg3[:], HT[0][:])
            nc.sync.dma_start(dbg_HT[:], dbg3[:])
            dbg4 = persist.tile([128, T], F32, name="dbg4")
            nc.vector.tensor_copy(dbg4[:], QTbf[0][:])
            nc.sync.dma_start(dbg_qt[:], dbg4[:])
            dbg5 = persist.tile([128, NKI * 16], F32, name="dbg5")
            nc.vector.tensor_copy(dbg5[:], p_nat[0][:])
            nc.sync.dma_start(dbg_pnat[:], dbg5[:])

        # ---------------- phase 3: o_proj ----------------
        with tc.tile_pool(name="ypsum", bufs=4, space="PSUM") as ypsum, \
             tc.tile_pool(name="ypool", bufs=3) as ypool:
            for tt in range(NKI):
                cs = slice(tt * 128, (tt + 1) * 128)
                ysb = ypool.tile([128, D], F32, name="ysb")
                for ec in range(2):
                    es = slice(ec * 512, (ec + 1) * 512)
                    yp = ypsum.tile([128, 512], F32, name="yp")
                    for p in range(2):
                        nc.tensor.matmul(yp[:], HT[p][:, cs], wo_sb[p][:, es],
                                         start=(p == 0), stop=(p == 1))
                    if ec == 0:
                        nc.scalar.copy(ysb[:, es], yp[:])
                    else:
                        nc.vector.tensor_copy(ysb[:, es], yp[:])
                nc.sync.dma_start(y_out[cs, :], ysb[:])

    nc.compile()
    return nc


_NC_CACHE = None


def _get_nc():
    global _NC_CACHE
    if _NC_CACHE is None:
        _NC_CACHE = build_nc()
    return _NC_CACHE


def _make_core_inputs(x, K_depth, V_depth, Wq, Wk, Wv, Wkd, Wvd, Wo, b, h):
    xT = np.ascontiguousarray(x[b].T)                       # (D, T)
    wq = Wq[:, h * G * HD:(h + 1) * G * HD]                 # (D, 256)
    wk = Wk[:, h * HD:(h + 1) * HD]
    wv = Wv[:, h * HD:(h + 1) * HD]
    wkd = Wkd[:, h * HD:(h + 1) * HD]
    wvd = Wvd[:, h * HD:(h + 1) * HD]
    # m0=[q0|q1] m1=[q2|q3] m2=[V|K] m3=[kd|vd]
    w_stack = np.concatenate([wq, wv, wk, wkd, wvd], axis=1)  # (D, 512)
    w_proj = np.ascontiguousarray(w_stack.reshape(NDC, 128, 512), dtype=np.float32)
    wo = np.ascontiguousarray(
        Wo[h * G * HD:(h + 1) * G * HD, :].reshape(2, 128, D)).astype(BF)
    kd = K_depth[b, h].reshape(T, LP, HD)
    kdT_l = np.transpose(kd, (1, 2, 0))                     # (LP, HD, T)
    kdT = np.concatenate([kdT_l, kdT_l], axis=1).astype(BF)  # (LP, 128, T)
    vd = V_depth[b, h].reshape(T, LP, HD)
    vd_dmaj = np.ascontiguousarray(
        np.transpose(vd, (0, 2, 1)).reshape(T, LP * HD)).astype(BF)
    return dict(xT=xT, w_proj=w_proj, wo=wo, kdT=np.ascontiguousarray(kdT),
                vd_dmaj=vd_dmaj)


def _const_inputs():
    k = np.arange(128)[:, None]
    q = np.arange(128)[None, :]
    trimask = (q >= k).astype(BF)                            # (128, 128)
    sel = np.zeros((LP, 128, 16), dtype=BF)
    for l in range(LP):
        sel[l, 0:64, l] = 1.0
        sel[l, 64:128, 8 + l] = 1.0
    densel = np.zeros((2, 16, 66), dtype=np.float32)
    for hp in range(2):
        densel[hp, hp * 8:(hp + 1) * 8, 64] = 1.0
    ones64 = np.ones((1, 64), dtype=np.float32)
    identity = np.eye(128, dtype=np.float32)
    return dict(trimask=trimask, sel=sel, densel=densel, ones64=ones64,
                identity=identity)


def _ensure_ntff_hook():
    """Provide the antenv.axon_hooks shim (missing in this image) so
    run_bass_kernel_spmd(trace=True) can capture NTFF profiles via the
    ctypes hook in libaxon_pjrt.so (mirrors trn_agent_boot.trn_boot)."""
    import types

    try:
        from antenv.axon_hooks import get_axon_ntff_profile_hook  # noqa: F401
        return
    except ImportError:
        pass
    import antenv

    mod = types.ModuleType("antenv.axon_hooks")
    mod._hook = None

    def set_axon_ntff_profile_hook(h):
        mod._hook = h

    def get_axon_ntff_profile_hook():
        return mod._hook

    mod.set_axon_ntff_profile_hook = set_axon_ntff_profile_hook
    mod.get_axon_ntff_profile_hook = get_axon_ntff_profile_hook
    sys.modules["antenv.axon_hooks"] = mod
    antenv.axon_hooks = mod

    so_path = "/opt/axon/libaxon_pjrt.so"
    try:
        sys.path.insert(0, "/root/.axon_site")
        from trn_agent_boot.trn_boot import _ntff_profile_via_ctypes
        hook = _ntff_profile_via_ctypes(so_path)
        if hook is not None:
            mod._hook = hook
    except Exception as e:  # degrade: tracing skipped, run still works
        print(f"ntff hook setup failed: {e}", file=sys.stderr)


def kernel(x, K_depth, V_depth, Wq, Wk, Wv, Wkd, Wvd, Wo, _trace=False):
    from concourse.bass_utils import run_bass_kernel_spmd

    if _trace:
        _ensure_ntff_hook()

    x = np.asarray(x, dtype=np.float32)
    K_depth = np.asarray(K_depth, dtype=np.float32)
    V_depth = np.asarray(V_depth, dtype=np.float32)
    Wq, Wk, Wv, Wkd, Wvd, Wo = (np.asarray(w, dtype=np.float32)
                                for w in (Wq, Wk, Wv, Wkd, Wvd, Wo))

    consts = _const_inputs()
    in_maps = []
    for core in range(8):
        b, h = divmod(core, HK)
        m = _make_core_inputs(x, K_depth, V_depth, Wq, Wk, Wv, Wkd, Wvd, Wo, b, h)
        m.update(consts)
        in_maps.append(m)

    nc = _get_nc()
    res = run_bass_kernel_spmd(nc, in_maps, list(range(8)), trace=_trace)
    kernel.last_result = res

    out = np.zeros((B, T, D), dtype=np.float32)
    k_write = np.zeros((B, HK, T, HD), dtype=np.float32)
    v_write = np.zeros((B, HK, T, HD), dtype=np.float32)
    for core in range(8):
        b, h = divmod(core, HK)
        out[b] += res.results[core]["y"]
        kvT = res.results[core]["kvT"]
        k_write[b, h] = kvT[0:64, :].T
        v_write[b, h] = kvT[64:128, :].T
    return out, k_write, v_write
